# revision 22
# baseline (speedup 1.0000x reference)
"""AttentionalSampler Trainium2 kernel.

Data-parallel over B*T=128 groups: 8 NeuronCores x 16 groups, processed as 8
pairs of groups per core. Per pair (2 groups stacked on 128 partitions where
useful):

  q path (fp32): tT (host pre-transposed) -> qproj -> RoPE -> LN -> qgT
  k path (fp16): mvT (host pre-transposed) -> kproj -> RoPE -> LN -> kz
                 -> PE transpose (kzT)
  attT[p,m] = kzT.T @ qgT per 128-patch chunk (fp16 matmul, fp32 accum,
              written directly in transposed orientation -> no attT transpose)
  softmax:  attE = exp(att - 2) [ACT, straight from PSUM] * ebias [DVE/GPS],
            where ebias = exp(2 - dist/8) is host-precomputed from positions.
  out = attE.T @ [mv | 1]: a ones-column appended to mv makes the final
        accumulating matmul produce the softmax row-sum for free; 1/sum is
        applied to the output as a per-partition scale.

Channel permutation [4g | 4g+1 | 4g+2 | 4g+3] is folded into the projection
weights so RoPE operates on contiguous free-dim blocks. rsqrt(var+eps) is
computed with a quadratic seed + 2 Newton iterations on DVE/GPSIMD, so the
ACT engine only ever uses the exp table set: zero table reloads in steady
state. ln_g (when scalar) and 1/sqrt(D) fold into the Newton output scale.
"""

import numpy as np
import ml_dtypes

D = 128
HP = 32
WP = 32
M = 64
B = 8
T = 16
P = HP * WP
BT = B * T
N_CORES = 8
BT_LOC = BT // N_CORES   # 16 groups per core
NPAIR = BT_LOC // 2      # 8 pairs per core
NC_CHUNK = P // 128      # 8 chunks of 128 patches per group
DECAY = 2.0
EPS = 1e-5
SQD = float(np.sqrt(np.float32(D)))
ESHIFT = 2.0             # exp(att-ESHIFT)*exp(ESHIFT-dist/8): fp16 headroom

# Newton rsqrt seed: minimax quadratic for u^-0.5 over u in [0.28, 3.2]
NEWT_C0 = 1.98420576
NEWT_C1 = -1.08812112
NEWT_C2 = 0.20749422

F32 = np.float32
FP16 = np.float16

# channel permutation: new j reads old perm[j]
PERM = np.concatenate([np.arange(0, D, 4), np.arange(1, D, 4),
                       np.arange(2, D, 4), np.arange(3, D, 4)])


def _host_tables():
    """Static (position-grid) tables shared by every core."""
    theta = (100.0 ** (-4.0 * np.arange(1, D // 4 + 1, dtype=np.float64) / D))
    # k-side RoPE tables in k-natural chunk layout [p'=128, c=8, 64]
    pidx = np.arange(P)
    h = (pidx // WP).astype(np.float64)   # patch row
    w = (pidx % WP).astype(np.float64)
    ch = np.cos(theta[None, :] * h[:, None])   # (P, 32)
    sh = np.sin(theta[None, :] * h[:, None])
    cw = np.cos(theta[None, :] * w[:, None])
    sw = np.sin(theta[None, :] * w[:, None])
    cck = np.concatenate([ch, cw], axis=1)          # (P, 64)
    ssk = np.concatenate([sh, -sw], axis=1)         # (P, 64)
    cck = cck.reshape(NC_CHUNK, 128, 64).transpose(1, 0, 2)  # (128, 8, 64)
    ssk = ssk.reshape(NC_CHUNK, 128, 64).transpose(1, 0, 2)
    # exp(ESHIFT - dist/8) lookup over coordinate deltas in [-31, 31]
    dd = np.arange(-(HP - 1), HP, dtype=np.float64)
    dist = np.sqrt(dd[:, None] ** 2 + dd[None, :] ** 2)
    etab = np.exp(ESHIFT - dist / (2.0 * DECAY ** 2))    # (63, 63)
    return theta, cck.astype(FP16), ssk.astype(FP16), etab


def _host_q_tables(theta, pos_loc):
    """Per-core RoPE tables from positions. pos_loc: (BT_LOC, M) int."""
    ph = (pos_loc // WP).astype(np.float64)
    pw = (pos_loc % WP).astype(np.float64)
    cq = np.concatenate([np.cos(theta[None, None, :] * ph[..., None]),
                         np.cos(theta[None, None, :] * pw[..., None])], -1)
    sq = np.concatenate([np.sin(theta[None, None, :] * ph[..., None]),
                         -np.sin(theta[None, None, :] * pw[..., None])], -1)
    cq = cq.reshape(NPAIR, 2 * M, 64)
    sq = sq.reshape(NPAIR, 2 * M, 64)
    return cq.astype(F32), sq.astype(F32)


def _host_ebias_T(etab, pos_loc):
    """exp(ESHIFT-dist/8) in transposed layout (NPAIR, 128p, 2g, 8c, 64m)."""
    ph = (pos_loc // WP).astype(np.int64)           # (BT_LOC, M)
    pw = (pos_loc % WP).astype(np.int64)
    pidx = np.arange(P)
    gh = pidx // WP
    gw = pidx % WP
    eb = etab[(ph[..., None] - gh) + (HP - 1),
              (pw[..., None] - gw) + (WP - 1)]      # (BT_LOC, M, P)
    ebT = (eb.reshape(NPAIR, 2, M, NC_CHUNK, 128)
             .transpose(0, 4, 1, 3, 2))             # (NPAIR, 128, 2, 8, 64)
    return np.ascontiguousarray(ebT.astype(FP16))


def _build_program(has_bq, has_bk, has_bln, has_g2):
    from contextlib import ExitStack
    import concourse.bass as bass
    import concourse.bacc as bacc
    import concourse.tile as tile
    import concourse.mybir as mybir

    dt = mybir.dt
    ALU = mybir.AluOpType
    ACTF = mybir.ActivationFunctionType
    AXL = mybir.AxisListType

    nc = bacc.Bacc("TRN2", target_bir_lowering=False)

    def din(name, shape, dtype):
        return nc.dram_tensor(name, shape, dtype, kind="ExternalInput").ap()

    tT_in = din("tT_in", [NPAIR, D, 2 * M], dt.float32)
    mvT_in = din("mvT_in", [NPAIR, D, 2 * NC_CHUNK, 128], dt.float16)
    mvx_in = din("mvx_in", [NPAIR, 128, 2 * NC_CHUNK, D + 1], dt.float16)
    eb_in = din("eb_in", [NPAIR, 128, 2, NC_CHUNK, M], dt.float16)
    wqt_in = din("wqt", [D, D], dt.float32)
    wkt_in = din("wkt", [D, D], dt.float16)
    cck_in = din("cck", [128, NC_CHUNK, 64], dt.float16)
    ssk_in = din("ssk", [128, NC_CHUNK, 64], dt.float16)
    ccq_in = din("ccq", [NPAIR, 2 * M, 64], dt.float32)
    ssq_in = din("ssq", [NPAIR, 2 * M, 64], dt.float32)
    idf_in = din("idf", [128, 128], dt.float32)
    idb_in = din("idb", [128, 128], dt.float16)
    nwt_in = din("nwt", [1, 4], dt.float32)   # [m0inv_k, A_k, m0inv_q, A_q]
    g2_in = din("g2v", [1, D], dt.float32) if has_g2 else None
    bg_in = din("bgv", [1, D], dt.float32) if has_bln else None
    gb_in = din("gbv", [1, D], dt.float32) if has_bln else None
    bq_in = din("bqv", [1, D], dt.float32) if has_bq else None
    bk_in = din("bkv", [1, D], dt.float32) if has_bk else None

    out_dram = nc.dram_tensor("out", [BT_LOC, M, D], dt.float32,
                              kind="ExternalOutput").ap()

    def bcast(dram_ap, parts=128):
        return bass.AP(tensor=dram_ap.tensor, offset=dram_ap.offset,
                       ap=[[0, parts]] + list(dram_ap.ap[1:]))

    with tile.TileContext(nc) as tc, ExitStack() as ctx:
        singles = ctx.enter_context(tc.tile_pool(name="singles", bufs=1))
        mvp = ctx.enter_context(tc.tile_pool(name="mvp", bufs=2))
        kp = ctx.enter_context(tc.tile_pool(name="kp", bufs=2))
        qp = ctx.enter_context(tc.tile_pool(name="qp", bufs=2))
        smal = ctx.enter_context(tc.tile_pool(name="smal", bufs=3))
        ps_w = ctx.enter_context(tc.tile_pool(name="ps_w", bufs=2, space="PSUM"))
        ps_b = ctx.enter_context(tc.tile_pool(name="ps_b", bufs=2, space="PSUM"))
        ps_big = ctx.enter_context(tc.tile_pool(name="ps_big", bufs=1, space="PSUM"))
        ps_out = ctx.enter_context(tc.tile_pool(name="ps_out", bufs=2, space="PSUM"))

        # ---- resident constants ----
        wqt = singles.tile([D, D], dt.float32)
        nc.sync.dma_start(out=wqt, in_=wqt_in)
        wkt = singles.tile([D, D], dt.float16)
        nc.sync.dma_start(out=wkt, in_=wkt_in)
        cck = singles.tile([128, NC_CHUNK, 64], dt.float16)
        nc.sync.dma_start(out=cck, in_=cck_in)
        ssk = singles.tile([128, NC_CHUNK, 64], dt.float16)
        nc.sync.dma_start(out=ssk, in_=ssk_in)
        idf = singles.tile([128, 128], dt.float32)
        nc.sync.dma_start(out=idf, in_=idf_in)
        idb = singles.tile([128, 128], dt.float16)
        nc.sync.dma_start(out=idb, in_=idb_in)
        nwt = singles.tile([128, 4], dt.float32)
        nc.sync.dma_start(out=nwt, in_=bcast(nwt_in))
        if has_g2:
            g2bc = singles.tile([128, D], dt.float32)
            nc.sync.dma_start(out=g2bc, in_=bcast(g2_in))
        if has_bln:
            bgbc = singles.tile([128, D], dt.float32)
            nc.sync.dma_start(out=bgbc, in_=bcast(bg_in))
            gbbc = singles.tile([128, D], dt.float32)
            nc.sync.dma_start(out=gbbc, in_=bcast(gb_in))
        if has_bq:
            bqbc = singles.tile([128, D], dt.float32)
            nc.sync.dma_start(out=bqbc, in_=bcast(bq_in))
        if has_bk:
            bkbc = singles.tile([128, D], dt.float32)
            nc.sync.dma_start(out=bkbc, in_=bcast(bk_in))

        m0k = nwt[:, 0:1]
        Ak = nwt[:, 1:2]
        m0q = nwt[:, 2:3]
        Aq = nwt[:, 3:4]
        eshift = singles.tile([128, 1], dt.float32)
        nc.vector.memset(eshift, -ESHIFT)

        def newton_rsqrt(eng, out, var, m0inv, A, nparts, tag):
            """out = A * u^-0.5 via quadratic seed + 2 Newton iters,
            u = (var+eps)*m0inv. All ops on `eng` (DVE or GPSIMD)."""
            cols = out.shape[-1]
            u = smal.tile([nparts, cols], dt.float32, tag=f"{tag}u")
            eng.tensor_scalar(out=u, in0=var, scalar1=EPS, scalar2=m0inv,
                              op0=ALU.add, op1=ALU.mult)
            w = smal.tile([nparts, cols], dt.float32, tag=f"{tag}w")
            eng.tensor_scalar(out=w, in0=u, scalar1=NEWT_C2, scalar2=NEWT_C1,
                              op0=ALU.mult, op1=ALU.add)
            eng.tensor_mul(w, w, u)
            eng.tensor_scalar_add(w, w, NEWT_C0)
            tmp = smal.tile([nparts, cols], dt.float32, tag=f"{tag}t")
            for _ in range(2):
                eng.tensor_mul(tmp, w, w)
                eng.tensor_mul(tmp, tmp, u)
                eng.tensor_scalar(out=tmp, in0=tmp, scalar1=-0.5, scalar2=1.5,
                                  op0=ALU.mult, op1=ALU.add)
                eng.tensor_mul(w, w, tmp)
            eng.tensor_scalar_mul(out, w, A)

        # ---- main loop over pairs ----
        for i in range(NPAIR):
            # loads
            tT = qp.tile([D, 2 * M], dt.float32, tag="tT")
            nc.sync.dma_start(out=tT, in_=tT_in[i])
            mvT = mvp.tile([128, 2 * NC_CHUNK, 128], dt.float16, tag="mvT")
            nc.sync.dma_start(out=mvT, in_=mvT_in[i])
            mvx = mvp.tile([128, 2 * NC_CHUNK, D + 1], dt.float16, tag="mvx")
            nc.sync.dma_start(out=mvx, in_=mvx_in[i])
            ebT = mvp.tile([128, 2, NC_CHUNK, M], dt.float16, tag="ebT")
            nc.sync.dma_start(out=ebT, in_=eb_in[i])
            ccq = qp.tile([2 * M, 64], dt.float32, tag="ccq")
            nc.sync.dma_start(out=ccq, in_=ccq_in[i])
            ssq = qp.tile([2 * M, 64], dt.float32, tag="ssq")
            nc.sync.dma_start(out=ssq, in_=ssq_in[i])

            # ---------------- q path (fp32) ----------------
            ps_q = ps_w.tile([128, 512], dt.float32, tag="ps")
            nc.tensor.matmul(ps_q[:, 0:128], tT, wqt, start=True, stop=True)
            q_f = qp.tile([2 * M, D], dt.float32, tag="q_f")
            nc.scalar.copy(out=q_f, in_=ps_q[:, 0:128])
            if has_bq:
                nc.vector.tensor_add(q_f, q_f, bqbc)
            # RoPE (permuted layout: [a c | b e] halves)
            ac = q_f[:, 0:64]
            be = q_f[:, 64:128]
            tq1 = qp.tile([2 * M, 64], dt.float32, tag="tq1")
            tq2 = qp.tile([2 * M, 64], dt.float32, tag="tq2")
            nc.vector.tensor_mul(tq1, be, ssq)
            nc.vector.tensor_mul(tq2, ac, ccq)
            nc.vector.tensor_sub(ac, tq2, tq1)
            nc.vector.tensor_mul(tq1, ac, ssq)
            nc.vector.tensor_mul(tq2, be, ccq)
            nc.vector.tensor_sub(be, tq2, tq1)
            # LN stats
            bnq = smal.tile([2 * M, 6], dt.float32, tag="bnq")
            nc.vector.bn_stats(out=bnq, in_=q_f)
            mvq = smal.tile([2 * M, 2], dt.float32, tag="mvq")
            nc.vector.bn_aggr(out=mvq, in_=bnq)
            # rstd_q (scaled by ln_g^2/sqrt(D) when foldable) on GPSIMD
            rstdq = smal.tile([2 * M, 1], dt.float32, tag="rstdq")
            newton_rsqrt(nc.gpsimd, rstdq, mvq[:, 1:2], m0q, Aq, 2 * M, "nq")
            qz = qp.tile([2 * M, D], dt.float32, tag="qz")
            nc.vector.tensor_scalar(out=qz, in0=q_f, scalar1=mvq[:, 0:1],
                                    scalar2=rstdq, op0=ALU.subtract,
                                    op1=ALU.mult)
            if has_g2:
                qg = qp.tile([2 * M, D], dt.float32, tag="qg")
                nc.vector.tensor_mul(qg, qz, g2bc)
            else:
                qg = qz
            if has_bln:
                nc.vector.tensor_add(qg, qg, bgbc)
                cexp = smal.tile([2 * M, 1], dt.float32, tag="cexp")
                trash = qp.tile([2 * M, D], dt.float32, tag="trash")
                nc.vector.tensor_tensor_reduce(
                    out=trash, in0=qz, in1=gbbc, scale=1.0, scalar=0.0,
                    op0=ALU.mult, op1=ALU.add, accum_out=cexp)
            ps_qg = ps_w.tile([128, 512], dt.float32, tag="ps")
            nc.tensor.transpose(ps_qg[:, 0:128], qg, idf)
            qgT = qp.tile([D, 2 * M], dt.float16, tag="qgT")
            nc.scalar.copy(out=qgT, in_=ps_qg[:, 0:128])
            # ---------------- k path (fp16) ----------------
            # kproj (mvT pre-transposed on host)
            k_b = kp.tile([128, 2 * NC_CHUNK, D], dt.float16, tag="k_b")
            for j in range(4):
                ps4 = ps_w.tile([128, 512], dt.float32, tag="ps")
                for cc in range(4):
                    c = 4 * j + cc
                    nc.tensor.matmul(ps4[:, cc * 128:(cc + 1) * 128],
                                     mvT[:, c, :], wkt, start=True, stop=True)
                nc.scalar.copy(out=k_b[:, 4 * j:4 * j + 4, :], in_=ps4)
            if has_bk:
                for c in range(2 * NC_CHUNK):
                    nc.vector.tensor_add(k_b[:, c, :], k_b[:, c, :], bkbc)
            # RoPE split: DVE chunks 0:11, GPSIMD chunks 11:16
            for eng, lo, hi in ((nc.vector, 0, 8), (nc.vector, 8, 11),
                                (nc.gpsimd, 11, 16)):
                n = hi - lo
                tlo = lo % NC_CHUNK
                ack = k_b[:, lo:hi, 0:64]
                bek = k_b[:, lo:hi, 64:128]
                cc_t = cck[:, tlo:tlo + n, :]
                ss_t = ssk[:, tlo:tlo + n, :]
                tk1 = kp.tile([128, n, 64], dt.float16, tag=f"tk1_{lo}")
                tk2 = kp.tile([128, n, 64], dt.float16, tag=f"tk2_{lo}")
                eng.tensor_mul(tk1, bek, ss_t)
                eng.tensor_mul(tk2, ack, cc_t)
                eng.tensor_sub(ack, tk2, tk1)
                eng.tensor_mul(tk1, ack, ss_t)
                eng.tensor_mul(tk2, bek, cc_t)
                eng.tensor_sub(bek, tk2, tk1)
            # LN stats per chunk
            bnk = kp.tile([128, 2 * NC_CHUNK, 6], dt.float32, tag="bnk")
            for c in range(2 * NC_CHUNK):
                nc.vector.bn_stats(out=bnk[:, c, :], in_=k_b[:, c, :])
            kmv = kp.tile([128, 2 * NC_CHUNK, 2], dt.float32, tag="kmv")
            for c in range(2 * NC_CHUNK):
                nc.vector.bn_aggr(out=kmv[:, c, :], in_=bnk[:, c, :])
            rstdk = smal.tile([128, 2 * NC_CHUNK], dt.float32, tag="rstdk")
            newton_rsqrt(nc.gpsimd, rstdk, kmv[:, :, 1], m0k, Ak, 128, "nk")
            # kz = k - mu (fp16); rstd_k folds into the per-chunk exp scale
            kz = kp.tile([128, 2 * NC_CHUNK, D], dt.float16, tag="kz")
            for c in range(10, 2 * NC_CHUNK):
                nc.gpsimd.tensor_scalar_sub(kz[:, c, :], k_b[:, c, :],
                                            kmv[:, c, 0:1])
            for c in range(10):
                nc.vector.tensor_scalar_sub(kz[:, c, :], k_b[:, c, :],
                                            kmv[:, c, 0:1])
            kzT = kp.tile([128, 2 * NC_CHUNK, D], dt.float16, tag="kzT")
            for j in range(4):
                ps4 = ps_b.tile([128, 512], dt.float16, tag="psb")
                for cc in range(4):
                    c = 4 * j + cc
                    nc.tensor.transpose(ps4[:, cc * 128:(cc + 1) * 128],
                                        kz[:, c, :], idb)
                nc.scalar.copy(out=kzT[:, 4 * j:4 * j + 4, :], in_=ps4)

            # ---------------- attention (transposed: [p, m]) ----------------
            att_ps = ps_big.tile([128, P], dt.float32, tag="big")
            for gi in range(2):
                rhs = qgT[:, gi * M:(gi + 1) * M]
                for c in range(NC_CHUNK):
                    o = (gi * NC_CHUNK + c) * M
                    nc.tensor.matmul(att_ps[:, o:o + M],
                                     kzT[:, gi * NC_CHUNK + c, :], rhs,
                                     start=True, stop=True)
            attE = kp.tile([128, 2 * NC_CHUNK * M], dt.float16, tag="attE")
            for c2 in range(2 * NC_CHUNK):
                o = c2 * M
                nc.scalar.activation(out=attE[:, o:o + M],
                                     in_=att_ps[:, o:o + M], func=ACTF.Exp,
                                     bias=eshift, scale=rstdk[:, c2:c2 + 1])
            ebf = ebT.rearrange("p g c m -> p (g c m)")
            nc.vector.tensor_mul(attE[:, 0:512], attE[:, 0:512],
                                 ebf[:, 0:512])
            nc.gpsimd.tensor_mul(attE[:, 512:1024], attE[:, 512:1024],
                                 ebf[:, 512:1024])
            # out = attE.T @ [mv | 1]: ones column gives softmax sum for free
            out_ps = ps_out.tile([128, D + 1], dt.float32, tag="out")
            for gi in range(2):
                for c in range(NC_CHUNK):
                    o = (gi * NC_CHUNK + c) * M
                    nc.tensor.matmul(
                        out_ps[gi * M:(gi + 1) * M, :],
                        attE[:, o:o + M],
                        mvx[:, gi * NC_CHUNK + c, :],
                        start=(c == 0), stop=(c == NC_CHUNK - 1))
            srec = smal.tile([128, 1], dt.float32, tag="srec")
            nc.vector.reciprocal(srec, out_ps[:, D:D + 1])
            out_f = smal.tile([128, D], dt.float32, tag="out_f")
            nc.vector.tensor_scalar_mul(out_f, out_ps[:, 0:D], srec)
            nc.sync.dma_start(
                out=out_dram[2 * i:2 * i + 2].rearrange("g m d -> (g m) d"),
                in_=out_f)

    nc.compile()
    return nc


_PROG_CACHE = {}


LAST_RESULT = None


def kernel(t, mv, positions, Wq, bq, Wk, bk, ln_g, ln_b, _trace=False):
    global LAST_RESULT
    from concourse.bass_utils import run_bass_kernel_spmd

    t = np.ascontiguousarray(np.asarray(t, F32).reshape(BT, M, D))
    mv_a = np.ascontiguousarray(np.asarray(mv, F32).reshape(BT, P, D).astype(FP16))
    pos = np.asarray(positions).reshape(BT, M).astype(np.int64)
    Wq = np.asarray(Wq, F32)
    Wk = np.asarray(Wk, F32)
    bq = np.asarray(bq, F32)
    bk = np.asarray(bk, F32)
    ln_g = np.asarray(ln_g, F32)
    ln_b = np.asarray(ln_b, F32)

    theta, cck, ssk, etab = _host_tables()

    wqt = np.ascontiguousarray(Wq.T[:, PERM].astype(F32))
    wkt = np.ascontiguousarray(Wk.T[:, PERM].astype(FP16))
    g_p = ln_g[PERM]
    b_p = ln_b[PERM]
    bq_p = bq[PERM].astype(F32)
    bk_p = bk[PERM].astype(F32)

    has_bq = bool(np.any(bq_p))
    has_bk = bool(np.any(bk_p))
    has_bln = bool(np.any(b_p))
    # scalar ln_g folds into the q-side rstd scale
    g_scalar = float(g_p[0])
    has_g2 = bool(np.any(np.abs(g_p - g_scalar) > 0))
    if has_g2:
        g2v = ((g_p * g_p / SQD).astype(F32))[None, :]
        q_scale = 1.0
    else:
        q_scale = g_scalar * g_scalar / SQD
    bgv = (b_p * g_p / SQD).astype(F32)[None, :]
    gbv = (g_p * b_p / SQD).astype(F32)[None, :]

    # Newton rsqrt normalization: m0 ~ E[var] = ||W||_F^2 / D
    m0_k = float((Wk.astype(np.float64) ** 2).sum() / D)
    m0_q = float((Wq.astype(np.float64) ** 2).sum() / D)
    nwt = np.array([[1.0 / m0_k, m0_k ** -0.5,
                     1.0 / m0_q, (m0_q ** -0.5) * q_scale]], dtype=F32)

    key = (has_bq, has_bk, has_bln, has_g2)
    if key not in _PROG_CACHE:
        _PROG_CACHE[key] = _build_program(*key)
    nc = _PROG_CACHE[key]

    idf = np.eye(128, dtype=F32)
    idb = np.eye(128, dtype=FP16)

    # host-side pre-transposes
    tT_all = np.ascontiguousarray(
        t.reshape(BT // 2, 2 * M, D).transpose(0, 2, 1))      # (BT/2, D, 2M)
    mvT_all = np.ascontiguousarray(
        mv_a.reshape(BT // 2, 2, NC_CHUNK, 128, D)
            .transpose(0, 4, 1, 2, 3)
            .reshape(BT // 2, D, 2 * NC_CHUNK, 128))
    # mv with a ones column appended: (BT/2, 128p, 16c, 129)
    mvx_all = np.empty((BT // 2, 128, 2 * NC_CHUNK, D + 1), dtype=FP16)
    mvx_all[..., :D] = (mv_a.reshape(BT // 2, 2, NC_CHUNK, 128, D)
                            .transpose(0, 3, 1, 2, 4)
                            .reshape(BT // 2, 128, 2 * NC_CHUNK, D))
    mvx_all[..., D] = 1.0

    in_maps = []
    for ci in range(N_CORES):
        sl = slice(ci * BT_LOC, (ci + 1) * BT_LOC)
        slp = slice(ci * NPAIR, (ci + 1) * NPAIR)
        ccq, ssq = _host_q_tables(theta, pos[sl])
        ebh = _host_ebias_T(etab, pos[sl])
        im = {
            "tT_in": tT_all[slp],
            "mvT_in": mvT_all[slp],
            "mvx_in": mvx_all[slp],
            "eb_in": ebh,
            "wqt": wqt, "wkt": wkt,
            "cck": np.ascontiguousarray(cck),
            "ssk": np.ascontiguousarray(ssk),
            "ccq": ccq, "ssq": ssq,
            "idf": idf, "idb": idb,
            "nwt": nwt,
        }
        if has_g2:
            im["g2v"] = g2v
        if has_bln:
            im["bgv"] = bgv
            im["gbv"] = gbv
        if has_bq:
            im["bqv"] = bq_p[None, :]
        if has_bk:
            im["bkv"] = bk_p[None, :]
        in_maps.append(im)

    res = run_bass_kernel_spmd(nc, in_maps, core_ids=list(range(N_CORES)),
                               trace=_trace)
    LAST_RESULT = res
    out = np.concatenate([r["out"].reshape(BT_LOC, M, D) for r in res.results])
    return out.reshape(B, T, M, D).astype(F32)


# revision 24
# speedup vs baseline: 1.6191x; 1.6191x over previous
"""AttentionalSampler Trainium2 kernel.

Data-parallel over B*T=128 groups: 8 NeuronCores x 16 groups, processed as 8
pairs of groups per core. Per pair (2 groups stacked on 128 partitions where
useful):

  q path (fp32): tT (host pre-transposed) -> qproj -> RoPE -> LN -> qgT
  k path (fp16): mvT (host pre-transposed) -> kproj -> RoPE -> LN -> kz
                 -> PE transpose (kzT)
  attT[p,m] = kzT.T @ qgT per 128-patch chunk (fp16 matmul, fp32 accum,
              written directly in transposed orientation -> no attT transpose)
  softmax:  attE = exp(att - 2) [ACT, straight from PSUM] * ebias [DVE/GPS],
            where ebias = exp(2 - dist/8) is host-precomputed from positions.
  out = attE.T @ [mv | 1]: a ones-column appended to mv makes the final
        accumulating matmul produce the softmax row-sum for free; 1/sum is
        applied to the output as a per-partition scale.

Channel permutation [4g | 4g+1 | 4g+2 | 4g+3] is folded into the projection
weights so RoPE operates on contiguous free-dim blocks. rsqrt(var+eps) is
computed with a quadratic seed + 2 Newton iterations on DVE/GPSIMD, so the
ACT engine only ever uses the exp table set: zero table reloads in steady
state. ln_g (when scalar) and 1/sqrt(D) fold into the Newton output scale.
"""

import numpy as np
import ml_dtypes

D = 128
HP = 32
WP = 32
M = 64
B = 8
T = 16
P = HP * WP
BT = B * T
N_CORES = 8
BT_LOC = BT // N_CORES   # 16 groups per core
NPAIR = BT_LOC // 2      # 8 pairs per core
NC_CHUNK = P // 128      # 8 chunks of 128 patches per group
DECAY = 2.0
EPS = 1e-5
SQD = float(np.sqrt(np.float32(D)))
ESHIFT = 2.0             # exp(att-ESHIFT)*exp(ESHIFT-dist/8): fp16 headroom

# Newton rsqrt seed: minimax quadratic for u^-0.5 over u in [0.28, 3.2]
NEWT_C0 = 1.98420576
NEWT_C1 = -1.08812112
NEWT_C2 = 0.20749422

F32 = np.float32
FP16 = np.float16

# channel permutation: new j reads old perm[j]
PERM = np.concatenate([np.arange(0, D, 4), np.arange(1, D, 4),
                       np.arange(2, D, 4), np.arange(3, D, 4)])


def _host_tables():
    """Static (position-grid) tables shared by every core."""
    theta = (100.0 ** (-4.0 * np.arange(1, D // 4 + 1, dtype=np.float64) / D))
    # k-side RoPE tables in k-natural chunk layout [p'=128, c=8, 64]
    pidx = np.arange(P)
    h = (pidx // WP).astype(np.float64)   # patch row
    w = (pidx % WP).astype(np.float64)
    ch = np.cos(theta[None, :] * h[:, None])   # (P, 32)
    sh = np.sin(theta[None, :] * h[:, None])
    cw = np.cos(theta[None, :] * w[:, None])
    sw = np.sin(theta[None, :] * w[:, None])
    cck = np.concatenate([ch, cw], axis=1)          # (P, 64)
    ssk = np.concatenate([sh, -sw], axis=1)         # (P, 64)
    cck = cck.reshape(NC_CHUNK, 128, 64).transpose(1, 0, 2)  # (128, 8, 64)
    ssk = ssk.reshape(NC_CHUNK, 128, 64).transpose(1, 0, 2)
    # exp(ESHIFT - dist/8) lookup over coordinate deltas in [-31, 31]
    dd = np.arange(-(HP - 1), HP, dtype=np.float64)
    dist = np.sqrt(dd[:, None] ** 2 + dd[None, :] ** 2)
    etab = np.exp(ESHIFT - dist / (2.0 * DECAY ** 2))    # (63, 63)
    return theta, cck.astype(FP16), ssk.astype(FP16), etab


def _host_q_tables(theta, pos_loc):
    """Per-core RoPE tables from positions. pos_loc: (BT_LOC, M) int."""
    ph = (pos_loc // WP).astype(np.float64)
    pw = (pos_loc % WP).astype(np.float64)
    cq = np.concatenate([np.cos(theta[None, None, :] * ph[..., None]),
                         np.cos(theta[None, None, :] * pw[..., None])], -1)
    sq = np.concatenate([np.sin(theta[None, None, :] * ph[..., None]),
                         -np.sin(theta[None, None, :] * pw[..., None])], -1)
    cq = cq.reshape(NPAIR, 2 * M, 64)
    sq = sq.reshape(NPAIR, 2 * M, 64)
    return cq.astype(F32), sq.astype(F32)


def _host_ebias_T(etab, pos_loc):
    """exp(ESHIFT-dist/8) in transposed layout (NPAIR, 128p, 2g, 8c, 64m)."""
    ph = (pos_loc // WP).astype(np.int64)           # (BT_LOC, M)
    pw = (pos_loc % WP).astype(np.int64)
    pidx = np.arange(P)
    gh = pidx // WP
    gw = pidx % WP
    eb = etab[(ph[..., None] - gh) + (HP - 1),
              (pw[..., None] - gw) + (WP - 1)]      # (BT_LOC, M, P)
    ebT = (eb.reshape(NPAIR, 2, M, NC_CHUNK, 128)
             .transpose(0, 4, 1, 3, 2))             # (NPAIR, 128, 2, 8, 64)
    return np.ascontiguousarray(ebT.astype(FP16))


def _build_program(has_bq, has_bk, has_bln, has_g2):
    from contextlib import ExitStack
    import concourse.bass as bass
    import concourse.bacc as bacc
    import concourse.tile as tile
    import concourse.mybir as mybir

    dt = mybir.dt
    ALU = mybir.AluOpType
    ACTF = mybir.ActivationFunctionType
    AXL = mybir.AxisListType

    nc = bacc.Bacc("TRN2", target_bir_lowering=False)

    def din(name, shape, dtype):
        return nc.dram_tensor(name, shape, dtype, kind="ExternalInput").ap()

    tT_in = din("tT_in", [NPAIR, D, 2 * M], dt.float32)
    mvT_in = din("mvT_in", [NPAIR, D, 2 * NC_CHUNK, 128], dt.float16)
    mvx_in = din("mvx_in", [NPAIR, 128, 2 * NC_CHUNK, D + 1], dt.float16)
    eb_in = din("eb_in", [NPAIR, 128, 2, NC_CHUNK, M], dt.float16)
    wqt_in = din("wqt", [D, D], dt.float32)
    wkt_in = din("wkt", [D, D], dt.float16)
    cck_in = din("cck", [128, NC_CHUNK, 64], dt.float16)
    ssk_in = din("ssk", [128, NC_CHUNK, 64], dt.float16)
    ccq_in = din("ccq", [NPAIR, 2 * M, 64], dt.float32)
    ssq_in = din("ssq", [NPAIR, 2 * M, 64], dt.float32)
    idf_in = din("idf", [128, 128], dt.float32)
    idb_in = din("idb", [128, 128], dt.float16)
    nwt_in = din("nwt", [1, 4], dt.float32)   # [m0inv_k, A_k, m0inv_q, A_q]
    g2_in = din("g2v", [1, D], dt.float32) if has_g2 else None
    bg_in = din("bgv", [1, D], dt.float32) if has_bln else None
    gb_in = din("gbv", [1, D], dt.float32) if has_bln else None
    bq_in = din("bqv", [1, D], dt.float32) if has_bq else None
    bk_in = din("bkv", [1, D], dt.float32) if has_bk else None

    out_dram = nc.dram_tensor("out", [BT_LOC, M, D], dt.float32,
                              kind="ExternalOutput").ap()

    def bcast(dram_ap, parts=128):
        return bass.AP(tensor=dram_ap.tensor, offset=dram_ap.offset,
                       ap=[[0, parts]] + list(dram_ap.ap[1:]))

    with tile.TileContext(nc) as tc, ExitStack() as ctx:
        singles = ctx.enter_context(tc.tile_pool(name="singles", bufs=1))
        mvp = ctx.enter_context(tc.tile_pool(name="mvp", bufs=2))
        kp = ctx.enter_context(tc.tile_pool(name="kp", bufs=2))
        qp = ctx.enter_context(tc.tile_pool(name="qp", bufs=2))
        smal = ctx.enter_context(tc.tile_pool(name="smal", bufs=3))
        ps_w = ctx.enter_context(tc.tile_pool(name="ps_w", bufs=2, space="PSUM"))
        ps_b = ctx.enter_context(tc.tile_pool(name="ps_b", bufs=2, space="PSUM"))
        ps_big = ctx.enter_context(tc.tile_pool(name="ps_big", bufs=1, space="PSUM"))
        ps_out = ctx.enter_context(tc.tile_pool(name="ps_out", bufs=2, space="PSUM"))

        # ---- resident constants ----
        wqt = singles.tile([D, D], dt.float32)
        nc.sync.dma_start(out=wqt, in_=wqt_in)
        wkt = singles.tile([D, D], dt.float16)
        nc.sync.dma_start(out=wkt, in_=wkt_in)
        cck = singles.tile([128, NC_CHUNK, 64], dt.float16)
        nc.sync.dma_start(out=cck, in_=cck_in)
        ssk = singles.tile([128, NC_CHUNK, 64], dt.float16)
        nc.sync.dma_start(out=ssk, in_=ssk_in)
        idf = singles.tile([128, 128], dt.float32)
        nc.sync.dma_start(out=idf, in_=idf_in)
        idb = singles.tile([128, 128], dt.float16)
        nc.sync.dma_start(out=idb, in_=idb_in)
        nwt = singles.tile([128, 4], dt.float32)
        nc.sync.dma_start(out=nwt, in_=bcast(nwt_in))
        if has_g2:
            g2bc = singles.tile([128, D], dt.float32)
            nc.sync.dma_start(out=g2bc, in_=bcast(g2_in))
        if has_bln:
            bgbc = singles.tile([128, D], dt.float32)
            nc.sync.dma_start(out=bgbc, in_=bcast(bg_in))
            gbbc = singles.tile([128, D], dt.float32)
            nc.sync.dma_start(out=gbbc, in_=bcast(gb_in))
        if has_bq:
            bqbc = singles.tile([128, D], dt.float32)
            nc.sync.dma_start(out=bqbc, in_=bcast(bq_in))
        if has_bk:
            bkbc = singles.tile([128, D], dt.float32)
            nc.sync.dma_start(out=bkbc, in_=bcast(bk_in))

        m0k = nwt[:, 0:1]
        Ak = nwt[:, 1:2]
        m0q = nwt[:, 2:3]
        Aq = nwt[:, 3:4]
        eshift = singles.tile([128, 1], dt.float32)
        nc.vector.memset(eshift, -ESHIFT)

        def newton_rsqrt(eng, out, var, m0inv, A, nparts, tag):
            """out = A * u^-0.5 via quadratic seed + 2 Newton iters,
            u = (var+eps)*m0inv. All ops on `eng` (DVE or GPSIMD)."""
            cols = out.shape[-1]
            u = smal.tile([nparts, cols], dt.float32, tag=f"{tag}u")
            eng.tensor_scalar(out=u, in0=var, scalar1=EPS, scalar2=m0inv,
                              op0=ALU.add, op1=ALU.mult)
            w = smal.tile([nparts, cols], dt.float32, tag=f"{tag}w")
            eng.tensor_scalar(out=w, in0=u, scalar1=NEWT_C2, scalar2=NEWT_C1,
                              op0=ALU.mult, op1=ALU.add)
            eng.tensor_mul(w, w, u)
            eng.tensor_scalar_add(w, w, NEWT_C0)
            tmp = smal.tile([nparts, cols], dt.float32, tag=f"{tag}t")
            for _ in range(2):
                eng.tensor_mul(tmp, w, w)
                eng.tensor_mul(tmp, tmp, u)
                eng.tensor_scalar(out=tmp, in0=tmp, scalar1=-0.5, scalar2=1.5,
                                  op0=ALU.mult, op1=ALU.add)
                eng.tensor_mul(w, w, tmp)
            eng.tensor_scalar_mul(out, w, A)

        # ---- main loop over pairs ----
        for i in range(NPAIR):
            # loads
            tT = qp.tile([D, 2 * M], dt.float32, tag="tT")
            nc.sync.dma_start(out=tT, in_=tT_in[i])
            mvT = mvp.tile([128, 2 * NC_CHUNK, 128], dt.float16, tag="mvT")
            nc.sync.dma_start(out=mvT, in_=mvT_in[i])
            mvx = mvp.tile([128, 2 * NC_CHUNK, D + 1], dt.float16, tag="mvx")
            nc.sync.dma_start(out=mvx, in_=mvx_in[i])
            ebT = mvp.tile([128, 2, NC_CHUNK, M], dt.float16, tag="ebT")
            nc.sync.dma_start(out=ebT, in_=eb_in[i])
            ccq = qp.tile([2 * M, 64], dt.float32, tag="ccq")
            nc.sync.dma_start(out=ccq, in_=ccq_in[i])
            ssq = qp.tile([2 * M, 64], dt.float32, tag="ssq")
            nc.sync.dma_start(out=ssq, in_=ssq_in[i])

            # ---------------- q path (fp32) ----------------
            ps_q = ps_w.tile([128, 512], dt.float32, tag="ps")
            nc.tensor.matmul(ps_q[:, 0:128], tT, wqt, start=True, stop=True)
            q_f = qp.tile([2 * M, D], dt.float32, tag="q_f")
            nc.scalar.copy(out=q_f, in_=ps_q[:, 0:128])
            if has_bq:
                nc.vector.tensor_add(q_f, q_f, bqbc)
            # RoPE (permuted layout: [a c | b e] halves)
            ac = q_f[:, 0:64]
            be = q_f[:, 64:128]
            tq1 = qp.tile([2 * M, 64], dt.float32, tag="tq1")
            tq2 = qp.tile([2 * M, 64], dt.float32, tag="tq2")
            nc.gpsimd.tensor_mul(tq1, be, ssq)
            nc.gpsimd.tensor_mul(tq2, ac, ccq)
            nc.gpsimd.tensor_sub(ac, tq2, tq1)
            nc.gpsimd.tensor_mul(tq1, ac, ssq)
            nc.gpsimd.tensor_mul(tq2, be, ccq)
            nc.gpsimd.tensor_sub(be, tq2, tq1)
            # LN stats
            bnq = smal.tile([2 * M, 6], dt.float32, tag="bnq")
            nc.vector.bn_stats(out=bnq, in_=q_f)
            mvq = smal.tile([2 * M, 2], dt.float32, tag="mvq")
            nc.vector.bn_aggr(out=mvq, in_=bnq)
            # rstd_q (scaled by ln_g^2/sqrt(D) when foldable) on GPSIMD
            rstdq = smal.tile([2 * M, 1], dt.float32, tag="rstdq")
            newton_rsqrt(nc.gpsimd, rstdq, mvq[:, 1:2], m0q, Aq, 2 * M, "nq")
            qz = qp.tile([2 * M, D], dt.float32, tag="qz")
            nc.vector.tensor_scalar(out=qz, in0=q_f, scalar1=mvq[:, 0:1],
                                    scalar2=rstdq, op0=ALU.subtract,
                                    op1=ALU.mult)
            if has_g2:
                qg = qp.tile([2 * M, D], dt.float32, tag="qg")
                nc.vector.tensor_mul(qg, qz, g2bc)
            else:
                qg = qz
            if has_bln:
                nc.vector.tensor_add(qg, qg, bgbc)
                cexp = smal.tile([2 * M, 1], dt.float32, tag="cexp")
                trash = qp.tile([2 * M, D], dt.float32, tag="trash")
                nc.vector.tensor_tensor_reduce(
                    out=trash, in0=qz, in1=gbbc, scale=1.0, scalar=0.0,
                    op0=ALU.mult, op1=ALU.add, accum_out=cexp)
            ps_qg = ps_w.tile([128, 512], dt.float32, tag="ps")
            nc.tensor.transpose(ps_qg[:, 0:128], qg, idf)
            qgT = qp.tile([D, 2 * M], dt.float16, tag="qgT")
            nc.scalar.copy(out=qgT, in_=ps_qg[:, 0:128])
            # ---------------- k path (fp16) ----------------
            # kproj (mvT pre-transposed on host)
            k_b = kp.tile([128, 2 * NC_CHUNK, D], dt.float16, tag="k_b")
            for j in range(4):
                ps4 = ps_w.tile([128, 512], dt.float32, tag="ps")
                for cc in range(4):
                    c = 4 * j + cc
                    nc.tensor.matmul(ps4[:, cc * 128:(cc + 1) * 128],
                                     mvT[:, c, :], wkt, start=True, stop=True)
                nc.scalar.copy(out=k_b[:, 4 * j:4 * j + 4, :], in_=ps4)
            if has_bk:
                for c in range(2 * NC_CHUNK):
                    nc.vector.tensor_add(k_b[:, c, :], k_b[:, c, :], bkbc)
            # RoPE split: DVE chunks 0:11, GPSIMD chunks 11:16
            for eng, lo, hi in ((nc.vector, 0, 8), (nc.vector, 8, 11),
                                (nc.gpsimd, 11, 16)):
                n = hi - lo
                tlo = lo % NC_CHUNK
                ack = k_b[:, lo:hi, 0:64]
                bek = k_b[:, lo:hi, 64:128]
                cc_t = cck[:, tlo:tlo + n, :]
                ss_t = ssk[:, tlo:tlo + n, :]
                tk1 = kp.tile([128, n, 64], dt.float16, tag=f"tk1_{lo}")
                tk2 = kp.tile([128, n, 64], dt.float16, tag=f"tk2_{lo}")
                eng.tensor_mul(tk1, bek, ss_t)
                eng.tensor_mul(tk2, ack, cc_t)
                eng.tensor_sub(ack, tk2, tk1)
                eng.tensor_mul(tk1, ack, ss_t)
                eng.tensor_mul(tk2, bek, cc_t)
                eng.tensor_sub(bek, tk2, tk1)
            # LN stats per chunk
            bnk = kp.tile([128, 2 * NC_CHUNK, 6], dt.float32, tag="bnk")
            for c in range(2 * NC_CHUNK):
                nc.vector.bn_stats(out=bnk[:, c, :], in_=k_b[:, c, :])
            kmv = kp.tile([128, 2 * NC_CHUNK, 2], dt.float32, tag="kmv")
            for c in range(2 * NC_CHUNK):
                nc.vector.bn_aggr(out=kmv[:, c, :], in_=bnk[:, c, :])
            rstdk = smal.tile([128, 2 * NC_CHUNK], dt.float32, tag="rstdk")
            newton_rsqrt(nc.vector, rstdk, kmv[:, :, 1], m0k, Ak, 128, "nk")
            nmr = smal.tile([128, 2 * NC_CHUNK], dt.float32, tag="nmr")
            nc.vector.tensor_mul(nmr, kmv[:, :, 0], rstdk)
            nc.vector.tensor_scalar_mul(nmr, nmr, -1.0)
            # kz = k*rstd - mu*rstd (fp16), then transpose
            # split DVE / GPSIMD / ACT (ACT: Identity(x*scale+bias))
            kz = kp.tile([128, 2 * NC_CHUNK, D], dt.float16, tag="kz")
            for c in range(2 * NC_CHUNK):
                if c < 5:
                    nc.vector.tensor_scalar(
                        out=kz[:, c, :], in0=k_b[:, c, :],
                        scalar1=rstdk[:, c:c + 1], scalar2=nmr[:, c:c + 1],
                        op0=ALU.mult, op1=ALU.add)
                elif c < 10:
                    nc.gpsimd.tensor_scalar(
                        out=kz[:, c, :], in0=k_b[:, c, :],
                        scalar1=rstdk[:, c:c + 1], scalar2=nmr[:, c:c + 1],
                        op0=ALU.mult, op1=ALU.add)
                else:
                    nc.scalar.activation(
                        out=kz[:, c, :], in_=k_b[:, c, :],
                        func=ACTF.Identity,
                        bias=nmr[:, c:c + 1], scale=rstdk[:, c:c + 1])
            kzT = kp.tile([128, 2 * NC_CHUNK, D], dt.float16, tag="kzT")
            for j in range(4):
                ps4 = ps_b.tile([128, 512], dt.float16, tag="psb")
                for cc in range(4):
                    c = 4 * j + cc
                    nc.tensor.transpose(ps4[:, cc * 128:(cc + 1) * 128],
                                        kz[:, c, :], idb)
                nc.scalar.copy(out=kzT[:, 4 * j:4 * j + 4, :], in_=ps4)

            # ---------------- attention (transposed: [p, m]) ----------------
            att_ps = ps_big.tile([128, P], dt.float32, tag="big")
            for gi in range(2):
                rhs = qgT[:, gi * M:(gi + 1) * M]
                for c in range(NC_CHUNK):
                    o = (gi * NC_CHUNK + c) * M
                    nc.tensor.matmul(att_ps[:, o:o + M],
                                     kzT[:, gi * NC_CHUNK + c, :], rhs,
                                     start=True, stop=True)
            attE = kp.tile([128, 2 * NC_CHUNK * M], dt.float16, tag="attE")
            if has_bln:
                nc.scalar.activation(out=attE, in_=att_ps, func=ACTF.Exp,
                                     bias=cexp, scale=1.0)
            else:
                nc.scalar.activation(out=attE, in_=att_ps, func=ACTF.Exp,
                                     bias=eshift, scale=1.0)
            ebf = ebT.rearrange("p g c m -> p (g c m)")
            nc.vector.tensor_mul(attE[:, 0:512], attE[:, 0:512],
                                 ebf[:, 0:512])
            nc.gpsimd.tensor_mul(attE[:, 512:1024], attE[:, 512:1024],
                                 ebf[:, 512:1024])
            # out = attE.T @ [mv | 1]: ones column gives softmax sum for free
            out_ps = ps_out.tile([128, D + 1], dt.float32, tag="out")
            for gi in range(2):
                for c in range(NC_CHUNK):
                    o = (gi * NC_CHUNK + c) * M
                    nc.tensor.matmul(
                        out_ps[gi * M:(gi + 1) * M, :],
                        attE[:, o:o + M],
                        mvx[:, gi * NC_CHUNK + c, :],
                        start=(c == 0), stop=(c == NC_CHUNK - 1))
            srec = smal.tile([128, 1], dt.float32, tag="srec")
            nc.vector.reciprocal(srec, out_ps[:, D:D + 1])
            out_f = smal.tile([128, D], dt.float32, tag="out_f")
            nc.vector.tensor_scalar_mul(out_f, out_ps[:, 0:D], srec)
            nc.sync.dma_start(
                out=out_dram[2 * i:2 * i + 2].rearrange("g m d -> (g m) d"),
                in_=out_f)

    nc.compile()
    return nc


_PROG_CACHE = {}


LAST_RESULT = None


def kernel(t, mv, positions, Wq, bq, Wk, bk, ln_g, ln_b, _trace=False):
    global LAST_RESULT
    from concourse.bass_utils import run_bass_kernel_spmd

    t = np.ascontiguousarray(np.asarray(t, F32).reshape(BT, M, D))
    mv_a = np.ascontiguousarray(np.asarray(mv, F32).reshape(BT, P, D).astype(FP16))
    pos = np.asarray(positions).reshape(BT, M).astype(np.int64)
    Wq = np.asarray(Wq, F32)
    Wk = np.asarray(Wk, F32)
    bq = np.asarray(bq, F32)
    bk = np.asarray(bk, F32)
    ln_g = np.asarray(ln_g, F32)
    ln_b = np.asarray(ln_b, F32)

    theta, cck, ssk, etab = _host_tables()

    wqt = np.ascontiguousarray(Wq.T[:, PERM].astype(F32))
    wkt = np.ascontiguousarray(Wk.T[:, PERM].astype(FP16))
    g_p = ln_g[PERM]
    b_p = ln_b[PERM]
    bq_p = bq[PERM].astype(F32)
    bk_p = bk[PERM].astype(F32)

    has_bq = bool(np.any(bq_p))
    has_bk = bool(np.any(bk_p))
    has_bln = bool(np.any(b_p))
    # scalar ln_g folds into the q-side rstd scale
    g_scalar = float(g_p[0])
    has_g2 = bool(np.any(np.abs(g_p - g_scalar) > 0))
    if has_g2:
        g2v = ((g_p * g_p / SQD).astype(F32))[None, :]
        q_scale = 1.0
    else:
        q_scale = g_scalar * g_scalar / SQD
    bgv = (b_p * g_p / SQD).astype(F32)[None, :]
    gbv = (g_p * b_p / SQD).astype(F32)[None, :]

    # Newton rsqrt normalization: m0 ~ E[var] = ||W||_F^2 / D
    m0_k = float((Wk.astype(np.float64) ** 2).sum() / D)
    m0_q = float((Wq.astype(np.float64) ** 2).sum() / D)
    nwt = np.array([[1.0 / m0_k, m0_k ** -0.5,
                     1.0 / m0_q, (m0_q ** -0.5) * q_scale]], dtype=F32)

    key = (has_bq, has_bk, has_bln, has_g2)
    if key not in _PROG_CACHE:
        _PROG_CACHE[key] = _build_program(*key)
    nc = _PROG_CACHE[key]

    idf = np.eye(128, dtype=F32)
    idb = np.eye(128, dtype=FP16)

    # host-side pre-transposes
    tT_all = np.ascontiguousarray(
        t.reshape(BT // 2, 2 * M, D).transpose(0, 2, 1))      # (BT/2, D, 2M)
    mvT_all = np.ascontiguousarray(
        mv_a.reshape(BT // 2, 2, NC_CHUNK, 128, D)
            .transpose(0, 4, 1, 2, 3)
            .reshape(BT // 2, D, 2 * NC_CHUNK, 128))
    # mv with a ones column appended: (BT/2, 128p, 16c, 129)
    mvx_all = np.empty((BT // 2, 128, 2 * NC_CHUNK, D + 1), dtype=FP16)
    mvx_all[..., :D] = (mv_a.reshape(BT // 2, 2, NC_CHUNK, 128, D)
                            .transpose(0, 3, 1, 2, 4)
                            .reshape(BT // 2, 128, 2 * NC_CHUNK, D))
    mvx_all[..., D] = 1.0

    in_maps = []
    for ci in range(N_CORES):
        sl = slice(ci * BT_LOC, (ci + 1) * BT_LOC)
        slp = slice(ci * NPAIR, (ci + 1) * NPAIR)
        ccq, ssq = _host_q_tables(theta, pos[sl])
        ebh = _host_ebias_T(etab, pos[sl])
        im = {
            "tT_in": tT_all[slp],
            "mvT_in": mvT_all[slp],
            "mvx_in": mvx_all[slp],
            "eb_in": ebh,
            "wqt": wqt, "wkt": wkt,
            "cck": np.ascontiguousarray(cck),
            "ssk": np.ascontiguousarray(ssk),
            "ccq": ccq, "ssq": ssq,
            "idf": idf, "idb": idb,
            "nwt": nwt,
        }
        if has_g2:
            im["g2v"] = g2v
        if has_bln:
            im["bgv"] = bgv
            im["gbv"] = gbv
        if has_bq:
            im["bqv"] = bq_p[None, :]
        if has_bk:
            im["bkv"] = bk_p[None, :]
        in_maps.append(im)

    res = run_bass_kernel_spmd(nc, in_maps, core_ids=list(range(N_CORES)),
                               trace=_trace)
    LAST_RESULT = res
    out = np.concatenate([r["out"].reshape(BT_LOC, M, D) for r in res.results])
    return out.reshape(B, T, M, D).astype(F32)


# revision 25
# speedup vs baseline: 1.7573x; 1.0853x over previous
"""AttentionalSampler Trainium2 kernel.

Data-parallel over B*T=128 groups: 8 NeuronCores x 16 groups, processed as 8
pairs of groups per core. Per pair (2 groups stacked on 128 partitions where
useful):

  q path (fp32): tT (host pre-transposed) -> qproj -> RoPE -> LN -> qgT
  k path (fp16): mvT (host pre-transposed) -> kproj -> RoPE -> LN -> kz
                 -> PE transpose (kzT)
  attT[p,m] = kzT.T @ qgT per 128-patch chunk (fp16 matmul, fp32 accum,
              written directly in transposed orientation -> no attT transpose)
  softmax:  attE = exp(att - 2) [ACT, straight from PSUM] * ebias [DVE/GPS],
            where ebias = exp(2 - dist/8) is host-precomputed from positions.
  out = attE.T @ [mv | 1]: a ones-column appended to mv makes the final
        accumulating matmul produce the softmax row-sum for free; 1/sum is
        applied to the output as a per-partition scale.

Channel permutation [4g | 4g+1 | 4g+2 | 4g+3] is folded into the projection
weights so RoPE operates on contiguous free-dim blocks. rsqrt(var+eps) is
computed with a quadratic seed + 2 Newton iterations on DVE/GPSIMD, so the
ACT engine only ever uses the exp table set: zero table reloads in steady
state. ln_g (when scalar) and 1/sqrt(D) fold into the Newton output scale.
"""

import numpy as np
import ml_dtypes

D = 128
HP = 32
WP = 32
M = 64
B = 8
T = 16
P = HP * WP
BT = B * T
N_CORES = 8
BT_LOC = BT // N_CORES   # 16 groups per core
NPAIR = BT_LOC // 2      # 8 pairs per core
NC_CHUNK = P // 128      # 8 chunks of 128 patches per group
DECAY = 2.0
EPS = 1e-5
SQD = float(np.sqrt(np.float32(D)))
ESHIFT = 2.0             # exp(att-ESHIFT)*exp(ESHIFT-dist/8): fp16 headroom

# Newton rsqrt seed: minimax quadratic for u^-0.5 over u in [0.28, 3.2]
NEWT_C0 = 1.98420576
NEWT_C1 = -1.08812112
NEWT_C2 = 0.20749422

F32 = np.float32
FP16 = np.float16

# channel permutation: new j reads old perm[j]
PERM = np.concatenate([np.arange(0, D, 4), np.arange(1, D, 4),
                       np.arange(2, D, 4), np.arange(3, D, 4)])


def _host_tables():
    """Static (position-grid) tables shared by every core."""
    theta = (100.0 ** (-4.0 * np.arange(1, D // 4 + 1, dtype=np.float64) / D))
    # k-side RoPE tables in k-natural chunk layout [p'=128, c=8, 64]
    pidx = np.arange(P)
    h = (pidx // WP).astype(np.float64)   # patch row
    w = (pidx % WP).astype(np.float64)
    ch = np.cos(theta[None, :] * h[:, None])   # (P, 32)
    sh = np.sin(theta[None, :] * h[:, None])
    cw = np.cos(theta[None, :] * w[:, None])
    sw = np.sin(theta[None, :] * w[:, None])
    cck = np.concatenate([ch, cw], axis=1)          # (P, 64)
    ssk = np.concatenate([sh, -sw], axis=1)         # (P, 64)
    cck = cck.reshape(NC_CHUNK, 128, 64).transpose(1, 0, 2)  # (128, 8, 64)
    ssk = ssk.reshape(NC_CHUNK, 128, 64).transpose(1, 0, 2)
    # exp(ESHIFT - dist/8) lookup over coordinate deltas in [-31, 31]
    dd = np.arange(-(HP - 1), HP, dtype=np.float64)
    dist = np.sqrt(dd[:, None] ** 2 + dd[None, :] ** 2)
    etab = np.exp(ESHIFT - dist / (2.0 * DECAY ** 2))    # (63, 63)
    return theta, cck.astype(FP16), ssk.astype(FP16), etab


def _host_q_tables(theta, pos_loc):
    """Per-core RoPE tables from positions. pos_loc: (BT_LOC, M) int."""
    ph = (pos_loc // WP).astype(np.float64)
    pw = (pos_loc % WP).astype(np.float64)
    cq = np.concatenate([np.cos(theta[None, None, :] * ph[..., None]),
                         np.cos(theta[None, None, :] * pw[..., None])], -1)
    sq = np.concatenate([np.sin(theta[None, None, :] * ph[..., None]),
                         -np.sin(theta[None, None, :] * pw[..., None])], -1)
    cq = cq.reshape(NPAIR, 2 * M, 64)
    sq = sq.reshape(NPAIR, 2 * M, 64)
    return cq.astype(F32), sq.astype(F32)


def _host_ebias_T(etab, pos_loc):
    """exp(ESHIFT-dist/8) in transposed layout (NPAIR, 128p, 2g, 8c, 64m)."""
    ph = (pos_loc // WP).astype(np.int64)           # (BT_LOC, M)
    pw = (pos_loc % WP).astype(np.int64)
    pidx = np.arange(P)
    gh = pidx // WP
    gw = pidx % WP
    eb = etab[(ph[..., None] - gh) + (HP - 1),
              (pw[..., None] - gw) + (WP - 1)]      # (BT_LOC, M, P)
    ebT = (eb.reshape(NPAIR, 2, M, NC_CHUNK, 128)
             .transpose(0, 4, 1, 3, 2))             # (NPAIR, 128, 2, 8, 64)
    return np.ascontiguousarray(ebT.astype(FP16))


def _build_program(has_bq, has_bk, has_bln, has_g2):
    from contextlib import ExitStack
    import concourse.bass as bass
    import concourse.bacc as bacc
    import concourse.tile as tile
    import concourse.mybir as mybir

    dt = mybir.dt
    ALU = mybir.AluOpType
    ACTF = mybir.ActivationFunctionType
    AXL = mybir.AxisListType

    nc = bacc.Bacc("TRN2", target_bir_lowering=False)

    def din(name, shape, dtype):
        return nc.dram_tensor(name, shape, dtype, kind="ExternalInput").ap()

    tT_in = din("tT_in", [NPAIR, D, 2 * M], dt.float32)
    mvT_in = din("mvT_in", [NPAIR, D, 2 * NC_CHUNK, 128], dt.float16)
    mvx_in = din("mvx_in", [NPAIR, 128, 2 * NC_CHUNK, D + 1], dt.float16)
    eb_in = din("eb_in", [NPAIR, 128, 2, NC_CHUNK, M], dt.float16)
    wqt_in = din("wqt", [D, D], dt.float32)
    wkt_in = din("wkt", [D, D], dt.float16)
    cck_in = din("cck", [128, NC_CHUNK, 64], dt.float16)
    ssk_in = din("ssk", [128, NC_CHUNK, 64], dt.float16)
    ccq_in = din("ccq", [NPAIR, 2 * M, 64], dt.float32)
    ssq_in = din("ssq", [NPAIR, 2 * M, 64], dt.float32)
    idf_in = din("idf", [128, 128], dt.float32)
    idb_in = din("idb", [128, 128], dt.float16)
    nwt_in = din("nwt", [1, 4], dt.float32)   # [m0inv_k, A_k, m0inv_q, A_q]
    g2_in = din("g2v", [1, D], dt.float32) if has_g2 else None
    bg_in = din("bgv", [1, D], dt.float32) if has_bln else None
    gb_in = din("gbv", [1, D], dt.float32) if has_bln else None
    bq_in = din("bqv", [1, D], dt.float32) if has_bq else None
    bk_in = din("bkv", [1, D], dt.float32) if has_bk else None

    out_dram = nc.dram_tensor("out", [BT_LOC, M, D], dt.float32,
                              kind="ExternalOutput").ap()

    def bcast(dram_ap, parts=128):
        return bass.AP(tensor=dram_ap.tensor, offset=dram_ap.offset,
                       ap=[[0, parts]] + list(dram_ap.ap[1:]))

    with tile.TileContext(nc) as tc, ExitStack() as ctx:
        singles = ctx.enter_context(tc.tile_pool(name="singles", bufs=1))
        mvp = ctx.enter_context(tc.tile_pool(name="mvp", bufs=2))
        kp = ctx.enter_context(tc.tile_pool(name="kp", bufs=2))
        qp = ctx.enter_context(tc.tile_pool(name="qp", bufs=2))
        smal = ctx.enter_context(tc.tile_pool(name="smal", bufs=3))
        ps_w = ctx.enter_context(tc.tile_pool(name="ps_w", bufs=2, space="PSUM"))
        ps_b = ctx.enter_context(tc.tile_pool(name="ps_b", bufs=2, space="PSUM"))
        ps_big = ctx.enter_context(tc.tile_pool(name="ps_big", bufs=1, space="PSUM"))
        ps_out = ctx.enter_context(tc.tile_pool(name="ps_out", bufs=2, space="PSUM"))

        # ---- resident constants ----
        wqt = singles.tile([D, D], dt.float32)
        nc.sync.dma_start(out=wqt, in_=wqt_in)
        wkt = singles.tile([D, D], dt.float16)
        nc.sync.dma_start(out=wkt, in_=wkt_in)
        cck = singles.tile([128, NC_CHUNK, 64], dt.float16)
        nc.sync.dma_start(out=cck, in_=cck_in)
        ssk = singles.tile([128, NC_CHUNK, 64], dt.float16)
        nc.sync.dma_start(out=ssk, in_=ssk_in)
        idf = singles.tile([128, 128], dt.float32)
        nc.sync.dma_start(out=idf, in_=idf_in)
        idb = singles.tile([128, 128], dt.float16)
        nc.sync.dma_start(out=idb, in_=idb_in)
        nwt = singles.tile([128, 4], dt.float32)
        nc.sync.dma_start(out=nwt, in_=bcast(nwt_in))
        if has_g2:
            g2bc = singles.tile([128, D], dt.float32)
            nc.sync.dma_start(out=g2bc, in_=bcast(g2_in))
        if has_bln:
            bgbc = singles.tile([128, D], dt.float32)
            nc.sync.dma_start(out=bgbc, in_=bcast(bg_in))
            gbbc = singles.tile([128, D], dt.float32)
            nc.sync.dma_start(out=gbbc, in_=bcast(gb_in))
        if has_bq:
            bqbc = singles.tile([128, D], dt.float32)
            nc.sync.dma_start(out=bqbc, in_=bcast(bq_in))
        if has_bk:
            bkbc = singles.tile([128, D], dt.float32)
            nc.sync.dma_start(out=bkbc, in_=bcast(bk_in))

        m0k = nwt[:, 0:1]
        Ak = nwt[:, 1:2]
        m0q = nwt[:, 2:3]
        Aq = nwt[:, 3:4]
        eshift = singles.tile([128, 1], dt.float32)
        nc.vector.memset(eshift, -ESHIFT)

        def newton_rsqrt(eng, out, var, m0inv, A, nparts, tag):
            """out = A * u^-0.5 via quadratic seed + 2 Newton iters,
            u = (var+eps)*m0inv. All ops on `eng` (DVE or GPSIMD)."""
            cols = out.shape[-1]
            u = smal.tile([nparts, cols], dt.float32, tag=f"{tag}u")
            eng.tensor_scalar(out=u, in0=var, scalar1=EPS, scalar2=m0inv,
                              op0=ALU.add, op1=ALU.mult)
            w = smal.tile([nparts, cols], dt.float32, tag=f"{tag}w")
            eng.tensor_scalar(out=w, in0=u, scalar1=NEWT_C2, scalar2=NEWT_C1,
                              op0=ALU.mult, op1=ALU.add)
            eng.tensor_mul(w, w, u)
            eng.tensor_scalar_add(w, w, NEWT_C0)
            tmp = smal.tile([nparts, cols], dt.float32, tag=f"{tag}t")
            for _ in range(2):
                eng.tensor_mul(tmp, w, w)
                eng.tensor_mul(tmp, tmp, u)
                eng.tensor_scalar(out=tmp, in0=tmp, scalar1=-0.5, scalar2=1.5,
                                  op0=ALU.mult, op1=ALU.add)
                eng.tensor_mul(w, w, tmp)
            eng.tensor_scalar_mul(out, w, A)

        # ---- main loop over pairs ----
        for i in range(NPAIR):
            # loads
            tT = qp.tile([D, 2 * M], dt.float32, tag="tT")
            nc.sync.dma_start(out=tT, in_=tT_in[i])
            mvT = mvp.tile([128, 2 * NC_CHUNK, 128], dt.float16, tag="mvT")
            nc.sync.dma_start(out=mvT, in_=mvT_in[i])
            mvx = mvp.tile([128, 2 * NC_CHUNK, D + 1], dt.float16, tag="mvx")
            nc.sync.dma_start(out=mvx, in_=mvx_in[i])
            ebT = mvp.tile([128, 2, NC_CHUNK, M], dt.float16, tag="ebT")
            nc.sync.dma_start(out=ebT, in_=eb_in[i])
            ccq = qp.tile([2 * M, 64], dt.float32, tag="ccq")
            nc.sync.dma_start(out=ccq, in_=ccq_in[i])
            ssq = qp.tile([2 * M, 64], dt.float32, tag="ssq")
            nc.sync.dma_start(out=ssq, in_=ssq_in[i])

            # ---------------- q path (fp32) ----------------
            ps_q = ps_w.tile([128, 512], dt.float32, tag="ps")
            nc.tensor.matmul(ps_q[:, 0:128], tT, wqt, start=True, stop=True)
            q_f = qp.tile([2 * M, D], dt.float32, tag="q_f")
            nc.scalar.copy(out=q_f, in_=ps_q[:, 0:128])
            if has_bq:
                nc.vector.tensor_add(q_f, q_f, bqbc)
            # RoPE (permuted layout: [a c | b e] halves)
            ac = q_f[:, 0:64]
            be = q_f[:, 64:128]
            tq1 = qp.tile([2 * M, 64], dt.float32, tag="tq1")
            tq2 = qp.tile([2 * M, 64], dt.float32, tag="tq2")
            nc.vector.tensor_mul(tq1, be, ssq)
            nc.vector.tensor_mul(tq2, ac, ccq)
            nc.vector.tensor_sub(ac, tq2, tq1)
            nc.vector.tensor_mul(tq1, ac, ssq)
            nc.vector.tensor_mul(tq2, be, ccq)
            nc.vector.tensor_sub(be, tq2, tq1)
            # LN stats
            bnq = smal.tile([2 * M, 6], dt.float32, tag="bnq")
            nc.vector.bn_stats(out=bnq, in_=q_f)
            mvq = smal.tile([2 * M, 2], dt.float32, tag="mvq")
            nc.vector.bn_aggr(out=mvq, in_=bnq)
            # rstd_q (scaled by ln_g^2/sqrt(D) when foldable) on GPSIMD
            rstdq = smal.tile([2 * M, 1], dt.float32, tag="rstdq")
            newton_rsqrt(nc.gpsimd, rstdq, mvq[:, 1:2], m0q, Aq, 2 * M, "nq")
            qz = qp.tile([2 * M, D], dt.float32, tag="qz")
            nc.vector.tensor_scalar(out=qz, in0=q_f, scalar1=mvq[:, 0:1],
                                    scalar2=rstdq, op0=ALU.subtract,
                                    op1=ALU.mult)
            if has_g2:
                qg = qp.tile([2 * M, D], dt.float32, tag="qg")
                nc.vector.tensor_mul(qg, qz, g2bc)
            else:
                qg = qz
            if has_bln:
                nc.vector.tensor_add(qg, qg, bgbc)
                cexp = smal.tile([2 * M, 1], dt.float32, tag="cexp")
                trash = qp.tile([2 * M, D], dt.float32, tag="trash")
                nc.vector.tensor_tensor_reduce(
                    out=trash, in0=qz, in1=gbbc, scale=1.0, scalar=0.0,
                    op0=ALU.mult, op1=ALU.add, accum_out=cexp)
            ps_qg = ps_w.tile([128, 512], dt.float32, tag="ps")
            nc.tensor.transpose(ps_qg[:, 0:128], qg, idf)
            qgT = qp.tile([D, 2 * M], dt.float16, tag="qgT")
            nc.scalar.copy(out=qgT, in_=ps_qg[:, 0:128])
            # ---------------- k path (fp16) ----------------
            # kproj (mvT pre-transposed on host)
            k_b = kp.tile([128, 2 * NC_CHUNK, D], dt.float16, tag="k_b")
            for j in range(4):
                ps4 = ps_w.tile([128, 512], dt.float32, tag="ps")
                for cc in range(4):
                    c = 4 * j + cc
                    nc.tensor.matmul(ps4[:, cc * 128:(cc + 1) * 128],
                                     mvT[:, c, :], wkt, start=True, stop=True)
                nc.scalar.copy(out=k_b[:, 4 * j:4 * j + 4, :], in_=ps4)
            if has_bk:
                for c in range(2 * NC_CHUNK):
                    nc.vector.tensor_add(k_b[:, c, :], k_b[:, c, :], bkbc)
            # RoPE split: DVE chunks 0:11, GPSIMD chunks 11:16
            for eng, lo, hi in ((nc.vector, 0, 8), (nc.vector, 8, 11),
                                (nc.gpsimd, 11, 16)):
                n = hi - lo
                tlo = lo % NC_CHUNK
                ack = k_b[:, lo:hi, 0:64]
                bek = k_b[:, lo:hi, 64:128]
                cc_t = cck[:, tlo:tlo + n, :]
                ss_t = ssk[:, tlo:tlo + n, :]
                tk1 = kp.tile([128, n, 64], dt.float16, tag=f"tk1_{lo}")
                tk2 = kp.tile([128, n, 64], dt.float16, tag=f"tk2_{lo}")
                eng.tensor_mul(tk1, bek, ss_t)
                eng.tensor_mul(tk2, ack, cc_t)
                eng.tensor_sub(ack, tk2, tk1)
                eng.tensor_mul(tk1, ack, ss_t)
                eng.tensor_mul(tk2, bek, cc_t)
                eng.tensor_sub(bek, tk2, tk1)
            # LN stats per chunk
            bnk = kp.tile([128, 2 * NC_CHUNK, 6], dt.float32, tag="bnk")
            for c in range(2 * NC_CHUNK):
                nc.vector.bn_stats(out=bnk[:, c, :], in_=k_b[:, c, :])
            kmv = kp.tile([128, 2 * NC_CHUNK, 2], dt.float32, tag="kmv")
            for c in range(2 * NC_CHUNK):
                nc.vector.bn_aggr(out=kmv[:, c, :], in_=bnk[:, c, :])
            rstdk = smal.tile([128, 2 * NC_CHUNK], dt.float32, tag="rstdk")
            newton_rsqrt(nc.vector, rstdk, kmv[:, :, 1], m0k, Ak, 128, "nk")
            nmr = smal.tile([128, 2 * NC_CHUNK], dt.float32, tag="nmr")
            nc.vector.tensor_mul(nmr, kmv[:, :, 0], rstdk)
            nc.vector.tensor_scalar_mul(nmr, nmr, -1.0)
            # kz = k*rstd - mu*rstd (fp16), then transpose
            kz = kp.tile([128, 2 * NC_CHUNK, D], dt.float16, tag="kz")
            for c in range(2 * NC_CHUNK):
                if c < 8:
                    nc.vector.tensor_scalar(
                        out=kz[:, c, :], in0=k_b[:, c, :],
                        scalar1=rstdk[:, c:c + 1], scalar2=nmr[:, c:c + 1],
                        op0=ALU.mult, op1=ALU.add)
                elif c < 12:
                    nc.gpsimd.tensor_scalar(
                        out=kz[:, c, :], in0=k_b[:, c, :],
                        scalar1=rstdk[:, c:c + 1], scalar2=nmr[:, c:c + 1],
                        op0=ALU.mult, op1=ALU.add)
                else:
                    nc.scalar.activation(
                        out=kz[:, c, :], in_=k_b[:, c, :],
                        func=ACTF.Identity,
                        bias=nmr[:, c:c + 1], scale=rstdk[:, c:c + 1])
            kzT = kp.tile([128, 2 * NC_CHUNK, D], dt.float16, tag="kzT")
            for j in range(4):
                ps4 = ps_b.tile([128, 512], dt.float16, tag="psb")
                for cc in range(4):
                    c = 4 * j + cc
                    nc.tensor.transpose(ps4[:, cc * 128:(cc + 1) * 128],
                                        kz[:, c, :], idb)
                nc.scalar.copy(out=kzT[:, 4 * j:4 * j + 4, :], in_=ps4)

            # ---------------- attention (transposed: [p, m]) ----------------
            att_ps = ps_big.tile([128, P], dt.float32, tag="big")
            for gi in range(2):
                rhs = qgT[:, gi * M:(gi + 1) * M]
                for c in range(NC_CHUNK):
                    o = (gi * NC_CHUNK + c) * M
                    nc.tensor.matmul(att_ps[:, o:o + M],
                                     kzT[:, gi * NC_CHUNK + c, :], rhs,
                                     start=True, stop=True)
            attE = kp.tile([128, 2 * NC_CHUNK * M], dt.float16, tag="attE")
            if has_bln:
                nc.scalar.activation(out=attE, in_=att_ps, func=ACTF.Exp,
                                     bias=cexp, scale=1.0)
            else:
                nc.scalar.activation(out=attE, in_=att_ps, func=ACTF.Exp,
                                     bias=eshift, scale=1.0)
            ebf = ebT.rearrange("p g c m -> p (g c m)")
            nc.vector.tensor_mul(attE[:, 0:512], attE[:, 0:512],
                                 ebf[:, 0:512])
            nc.gpsimd.tensor_mul(attE[:, 512:1024], attE[:, 512:1024],
                                 ebf[:, 512:1024])
            # out = attE.T @ [mv | 1]: ones column gives softmax sum for free
            out_ps = ps_out.tile([128, D + 1], dt.float32, tag="out")
            for gi in range(2):
                for c in range(NC_CHUNK):
                    o = (gi * NC_CHUNK + c) * M
                    nc.tensor.matmul(
                        out_ps[gi * M:(gi + 1) * M, :],
                        attE[:, o:o + M],
                        mvx[:, gi * NC_CHUNK + c, :],
                        start=(c == 0), stop=(c == NC_CHUNK - 1))
            srec = smal.tile([128, 1], dt.float32, tag="srec")
            nc.vector.reciprocal(srec, out_ps[:, D:D + 1])
            out_f = smal.tile([128, D], dt.float32, tag="out_f")
            nc.vector.tensor_scalar_mul(out_f, out_ps[:, 0:D], srec)
            nc.sync.dma_start(
                out=out_dram[2 * i:2 * i + 2].rearrange("g m d -> (g m) d"),
                in_=out_f)

    nc.compile()
    return nc


_PROG_CACHE = {}


LAST_RESULT = None


def kernel(t, mv, positions, Wq, bq, Wk, bk, ln_g, ln_b, _trace=False):
    global LAST_RESULT
    from concourse.bass_utils import run_bass_kernel_spmd

    t = np.ascontiguousarray(np.asarray(t, F32).reshape(BT, M, D))
    mv_a = np.ascontiguousarray(np.asarray(mv, F32).reshape(BT, P, D).astype(FP16))
    pos = np.asarray(positions).reshape(BT, M).astype(np.int64)
    Wq = np.asarray(Wq, F32)
    Wk = np.asarray(Wk, F32)
    bq = np.asarray(bq, F32)
    bk = np.asarray(bk, F32)
    ln_g = np.asarray(ln_g, F32)
    ln_b = np.asarray(ln_b, F32)

    theta, cck, ssk, etab = _host_tables()

    wqt = np.ascontiguousarray(Wq.T[:, PERM].astype(F32))
    wkt = np.ascontiguousarray(Wk.T[:, PERM].astype(FP16))
    g_p = ln_g[PERM]
    b_p = ln_b[PERM]
    bq_p = bq[PERM].astype(F32)
    bk_p = bk[PERM].astype(F32)

    has_bq = bool(np.any(bq_p))
    has_bk = bool(np.any(bk_p))
    has_bln = bool(np.any(b_p))
    # scalar ln_g folds into the q-side rstd scale
    g_scalar = float(g_p[0])
    has_g2 = bool(np.any(np.abs(g_p - g_scalar) > 0))
    if has_g2:
        g2v = ((g_p * g_p / SQD).astype(F32))[None, :]
        q_scale = 1.0
    else:
        q_scale = g_scalar * g_scalar / SQD
    bgv = (b_p * g_p / SQD).astype(F32)[None, :]
    gbv = (g_p * b_p / SQD).astype(F32)[None, :]

    # Newton rsqrt normalization: m0 ~ E[var] = ||W||_F^2 / D
    m0_k = float((Wk.astype(np.float64) ** 2).sum() / D)
    m0_q = float((Wq.astype(np.float64) ** 2).sum() / D)
    nwt = np.array([[1.0 / m0_k, m0_k ** -0.5,
                     1.0 / m0_q, (m0_q ** -0.5) * q_scale]], dtype=F32)

    key = (has_bq, has_bk, has_bln, has_g2)
    if key not in _PROG_CACHE:
        _PROG_CACHE[key] = _build_program(*key)
    nc = _PROG_CACHE[key]

    idf = np.eye(128, dtype=F32)
    idb = np.eye(128, dtype=FP16)

    # host-side pre-transposes
    tT_all = np.ascontiguousarray(
        t.reshape(BT // 2, 2 * M, D).transpose(0, 2, 1))      # (BT/2, D, 2M)
    mvT_all = np.ascontiguousarray(
        mv_a.reshape(BT // 2, 2, NC_CHUNK, 128, D)
            .transpose(0, 4, 1, 2, 3)
            .reshape(BT // 2, D, 2 * NC_CHUNK, 128))
    # mv with a ones column appended: (BT/2, 128p, 16c, 129)
    mvx_all = np.empty((BT // 2, 128, 2 * NC_CHUNK, D + 1), dtype=FP16)
    mvx_all[..., :D] = (mv_a.reshape(BT // 2, 2, NC_CHUNK, 128, D)
                            .transpose(0, 3, 1, 2, 4)
                            .reshape(BT // 2, 128, 2 * NC_CHUNK, D))
    mvx_all[..., D] = 1.0

    in_maps = []
    for ci in range(N_CORES):
        sl = slice(ci * BT_LOC, (ci + 1) * BT_LOC)
        slp = slice(ci * NPAIR, (ci + 1) * NPAIR)
        ccq, ssq = _host_q_tables(theta, pos[sl])
        ebh = _host_ebias_T(etab, pos[sl])
        im = {
            "tT_in": tT_all[slp],
            "mvT_in": mvT_all[slp],
            "mvx_in": mvx_all[slp],
            "eb_in": ebh,
            "wqt": wqt, "wkt": wkt,
            "cck": np.ascontiguousarray(cck),
            "ssk": np.ascontiguousarray(ssk),
            "ccq": ccq, "ssq": ssq,
            "idf": idf, "idb": idb,
            "nwt": nwt,
        }
        if has_g2:
            im["g2v"] = g2v
        if has_bln:
            im["bgv"] = bgv
            im["gbv"] = gbv
        if has_bq:
            im["bqv"] = bq_p[None, :]
        if has_bk:
            im["bkv"] = bk_p[None, :]
        in_maps.append(im)

    res = run_bass_kernel_spmd(nc, in_maps, core_ids=list(range(N_CORES)),
                               trace=_trace)
    LAST_RESULT = res
    out = np.concatenate([r["out"].reshape(BT_LOC, M, D) for r in res.results])
    return out.reshape(B, T, M, D).astype(F32)
